# revision 28
# baseline (speedup 1.0000x reference)
"""Trainium2 Bass kernel for nn_Attention_55233279426826 (block-causal attention).

Reference computation (per batch b):
    xn = LayerNorm(x[b]) * gamma + beta
    q,k,v = split(xn @ w_qkv), 12 heads x 64
    attn  = softmax(block-causal-masked(q k^T / 8))
    out[b] = (attn v) @ w_out + b_out

Sharding (8 cores): batch (2) x head-group (4, 3 heads each).  Each core gets
its batch's x, the w_qkv columns and w_out rows of its 3 heads, and produces a
partial [2048, 768] output.  Host sums the 4 head-group partials per batch and
adds b_out.  gamma is folded into w_qkv on host; beta@w_qkv is a host-computed
per-channel bias added at QKV psum eviction.  Weights ship as bf16.

Per-core device program:
  1. LayerNorm stats in [token, dim] layout (bn_stats/bn_aggr), apply
     (x - mu) * rstd on DVE -> bf16 xn.
  2. xn -> xnT [768, 2048] via XBAR DMA transposes (off the PE).
  3. qkvT [576, 2048] = w_qkv^T @ xnT on PE; head strips laid out so the two
     heads of a score pair sit on opposite partition halves (auto row-tiling:
     two 64x128 score matmuls run concurrently in the PE array).  Head 2's
     q/k strips are duplicated to the hi partition half via SBUF DMA so odd-J
     score matmuls pair with even-J ones.
  4. v re-transposed to [keys, 64] via XBAR DMA, augmented with a ones column
     (index 64) so A@V also produces softmax denominators in psum row 64.
  5. Scores S_T[j, q] per 128-key block J / 512-query chunk c, exp on ACT
     (bounded scores, no max pass), masked corners zeroed, A@V into psum.
     1/den via DVE reciprocal + GPSIMD partition_broadcast (no PE involved),
     divided out during psum->SBUF eviction.
  6. Attention outputs packed into two [128, T] tiles ([h0;h1] and [h2;0]) so
     the out-projection runs as K=128 full-array matmuls; streamed to DRAM.
  7. QKV/LN/transpose work for token group g+1 is interleaved into attention
     chunk c=g so PE, ACT, DVE and DMA all stay busy.
"""

import contextlib
import ctypes
import math
import os
import sys
import types

import numpy as np

B = 2
T = 2048
D = 768
NPATCH = 64
HEADS = 12
DH = 64
NH = 3          # heads per core
CH = 3 * NH * DH  # 576 qkv channels per core
LN_EPS = 1e-5
NCORES = 8

_CACHE = {}


def _install_axon_hooks_shim():
    """This image's antenv lacks axon_hooks; synthesize it so that
    run_bass_kernel_spmd(trace=True) finds the NTFF profile hook instead of
    crashing on import.  Safe no-op if profiling symbols are unavailable."""
    if "antenv.axon_hooks" in sys.modules:
        return
    mod = types.ModuleType("antenv.axon_hooks")
    _hook = [None]
    mod.set_axon_ntff_profile_hook = lambda h: _hook.__setitem__(0, h)
    mod.get_axon_ntff_profile_hook = lambda: _hook[0]
    sys.modules["antenv.axon_hooks"] = mod
    try:
        lib = ctypes.CDLL("/opt/axon/libaxon_pjrt.so")
        if not hasattr(lib, "axon_start_nrt_profile"):
            return
        lib.axon_start_nrt_profile.argtypes = [
            ctypes.POINTER(ctypes.c_int64),
            ctypes.c_size_t,
        ]
        lib.axon_start_nrt_profile.restype = ctypes.c_int64
        lib.axon_stop_nrt_profile.argtypes = [ctypes.c_char_p]
        lib.axon_stop_nrt_profile.restype = ctypes.c_int64

        @contextlib.contextmanager
        def _hook_cm(output_dir, device_ids):
            import jax

            jax.devices()
            if device_ids:
                ids = (ctypes.c_int64 * len(device_ids))(*device_ids)
                rc = lib.axon_start_nrt_profile(ids, len(device_ids))
            else:
                rc = lib.axon_start_nrt_profile(None, 0)
            if rc != 0:
                raise RuntimeError(f"axon_start_nrt_profile rc={rc}")
            try:
                yield
            finally:
                n = lib.axon_stop_nrt_profile(str(output_dir).encode())
                print(f"profile: {n} file(s) -> {output_dir}", file=sys.stderr)

        mod.set_axon_ntff_profile_hook(_hook_cm)
    except OSError:
        pass


def _install_drain_split():
    """The walrus build in this container accepts only ONE sync wait per
    CTRL(drain) instruction; Tile's tail drain carries several.  Split the
    waits across a chain of drains."""
    import bass_rust
    import concourse.tile as tile
    from concourse.vector_clock import ScopedClock

    if getattr(tile.TileContext, "_drain_split_installed", False):
        return

    def _drain_and_barrier(self, tick_clock, wait_clock):
        nc = self.nc
        drain_inst = nc.sync.drain()
        wait_clock.add_sem_waits(
            drain_inst.ins, ScopedClock({None: tick_clock.global_clock})
        )
        si = drain_inst.ins.sync_info
        if si is not None:
            waits = list(si.on_wait)
            if len(waits) > 1:
                si.on_wait = waits[:1]
                for w in waits[1:]:
                    extra = nc.sync.drain()
                    extra.ins.sync_info = bass_rust.SyncInfo(
                        on_wait=[w], on_update=[]
                    )
        nc.all_engine_barrier()
        popped = nc._tile_sem_poison_stack.pop()
        assert popped is self._sem_poison
        nc.clear_and_free_semaphores(list(self.sems.allocated().values()))
        nc.all_engine_barrier()

    tile.TileContext._drain_and_barrier = _drain_and_barrier

    # Generic pass: walrus here allows 1 sync wait per instruction; move
    # extra waits onto nofuse NOPs inserted just before, on the same engine.
    from concourse import mybir

    orig_lower = tile.TileContext._lower_ordered_insts

    def _lower_split(self, ordered):
        for insts in ordered.values():
            new = []
            for inst in insts:
                si = getattr(inst, "sync_info", None)
                eng = getattr(inst, "engine", None)
                if si is not None and eng is not None:
                    waits = list(si.on_wait)
                    if len(waits) > 1:
                        movable = [w for w in waits
                                   if getattr(w, "sync_type", "") == "semaphore"]
                        keep = [w for w in waits if w not in movable]
                        if not keep:
                            keep = [movable.pop()]
                        for k, w in enumerate(movable):
                            nop = mybir.InstNoOp(
                                name=f"{inst.name}-wsplit{k}",
                                sync_info=mybir.SyncInfo(
                                    on_wait=[w], on_update=[]
                                ),
                                bass_nofuse=True,
                                engine=eng,
                            )
                            new.append(nop)
                        inst.sync_info = mybir.SyncInfo(
                            on_wait=keep, on_update=list(si.on_update)
                        )
                new.append(inst)
            insts[:] = new
        return orig_lower(self, ordered)

    tile.TileContext._lower_ordered_insts = _lower_split
    tile.TileContext._drain_split_installed = True


# qkvT row layout over six [128, T] tiles (64-row strips):
# t0 = [q0; q1], t1 = [k0; k1], t2 = [q2; v0], t3 = [k2; v1],
# t4 = [v2; k2copy], t5 = [--; q2copy]
# q and k of the same head share a partition offset (matmul operands must have
# equal base partitions); the head-2 hi copies land via SBUF->SBUF DMA.
Q_LOC = [(0, 0), (0, 64), (2, 0)]
K_LOC = [(1, 0), (1, 64), (3, 0)]
V_LOC = [(2, 64), (3, 64), (4, 0)]
Q2C = (5, 64)
K2C = (4, 64)
# host column order of the permuted per-core w_qkv (64-col segments)
SEG_ORDER = [("q", 0), ("q", 1), ("k", 0), ("k", 1), ("q", 2), ("v", 0),
             ("k", 2), ("v", 1), ("v", 2)]

C_CHUNKS = [(0, 128), (128, 128), (256, 128), (384, 128), (512, 64)]


def build_nc():
    import concourse.bass as bass
    import concourse.tile as tile
    from concourse import mybir

    _install_drain_split()

    f32 = mybir.dt.float32
    bf16 = mybir.dt.bfloat16
    AF = mybir.ActivationFunctionType
    Alu = mybir.AluOpType

    debug = bool(int(os.environ.get("KERNEL_DEBUG", "0")))
    nc = bass.Bass()
    x_d = nc.dram_tensor("x", [T, D], f32, kind="ExternalInput")
    wqkv_d = nc.dram_tensor("wqkv", [D, CH], bf16, kind="ExternalInput")
    woutp_d = nc.dram_tensor("woutp", [256, D], bf16, kind="ExternalInput")
    bw_d = nc.dram_tensor("bw", [640], f32, kind="ExternalInput")
    ident_d = nc.dram_tensor("ident", [128, 128], bf16, kind="ExternalInput")
    out_d = nc.dram_tensor("out", [T, D], f32, kind="ExternalOutput")
    if debug:
        dbg_qkvT_d = nc.dram_tensor("dbg_qkvT", [6, 128, T], bf16,
                                    kind="ExternalOutput")
        dbg_vaug_d = nc.dram_tensor("dbg_vaug", [NH, 128, 16, 128], bf16,
                                    kind="ExternalOutput")
        dbg_ocat_d = nc.dram_tensor("dbg_ocat", [2, 128, T], bf16,
                                    kind="ExternalOutput")
        dbg_den_d = nc.dram_tensor("dbg_den", [4, NH, 1, 512], f32,
                                   kind="ExternalOutput")
        dbg_rec_d = nc.dram_tensor("dbg_rec", [4, NH, 1, 512], f32,
                                   kind="ExternalOutput")

    with contextlib.ExitStack() as ctx:
        ctx.enter_context(
            nc.allow_low_precision(reason="bf16 PE inputs are intentional")
        )
        tc = ctx.enter_context(tile.TileContext(nc))
        consts = ctx.enter_context(tc.tile_pool(name="consts", bufs=1))
        wpool = ctx.enter_context(tc.tile_pool(name="w", bufs=1))
        qkvT_pool = ctx.enter_context(tc.tile_pool(name="qkvT", bufs=1))
        vaug_pool = ctx.enter_context(tc.tile_pool(name="vaug", bufs=1))
        ocat_pool = ctx.enter_context(tc.tile_pool(name="ocat", bufs=1))
        xpool = ctx.enter_context(tc.tile_pool(name="xin", bufs=1))
        xn_pool = ctx.enter_context(tc.tile_pool(name="xn", bufs=1))
        xnT_pool = ctx.enter_context(tc.tile_pool(name="xnT", bufs=1))
        io_pool = ctx.enter_context(tc.tile_pool(name="io", bufs=3))
        stats = ctx.enter_context(tc.tile_pool(name="stats", bufs=4))
        # bufs=2: group g+1's exp (emitted before group g's deferred A@V)
        # must not alias the pt tiles that A@V still reads
        pt_pool = ctx.enter_context(tc.tile_pool(name="pt", bufs=2))
        rec_pool = ctx.enter_context(tc.tile_pool(name="rec", bufs=2))
        tmp_pool = ctx.enter_context(tc.tile_pool(name="tmp", bufs=2))
        # PSUM banks: 2 (qkv/out-proj shared, tags mm0/mm1) + 3 (scores)
        # + 3 (attn out, tags ot0-2) = 8
        mm_ps = ctx.enter_context(tc.tile_pool(name="mm_ps", bufs=1, space="PSUM"))
        st_ps = ctx.enter_context(tc.tile_pool(name="st_ps", bufs=3, space="PSUM"))
        ot_ps = ctx.enter_context(tc.tile_pool(name="ot_ps", bufs=1, space="PSUM"))

        id_bf = consts.tile([128, 128], bf16, tag="idbf")
        nc.sync.dma_start(id_bf, ident_d[:, :])
        eps_t = consts.tile([128, 1], f32, tag="eps")
        nc.vector.memset(eps_t, LN_EPS)
        ones_t = consts.tile([128, DH], bf16, tag="ones")
        nc.vector.memset(ones_t.bitcast(bf16), 1.0)

        # DMA order: group-0 x tiles first (LN gates the pipeline), then
        # weights (QKV needs them by ~15us), then the rest of x.  Everything
        # split 4-way so no tile's arrival is bound to one ~20GB/s queue.
        x_sb = [xpool.tile([128, D], f32, tag=f"x{i}", name=f"xx{i}")
                for i in range(16)]

        def dma_x(i):
            for q in range(4):
                nc.sync.dma_start(
                    x_sb[i][:, 192 * q : 192 * (q + 1)],
                    x_d[128 * i : 128 * (i + 1), 192 * q : 192 * (q + 1)],
                )

        for i in range(4):
            dma_x(i)
        bw_sb = consts.tile([128, 5], f32, tag="bw")
        nc.sync.dma_start(bw_sb, bw_d[:].rearrange("(a p) -> p a", p=128))
        w_sb = []
        for j in range(6):
            wt = wpool.tile([128, CH], bf16, tag=f"w{j}", name=f"w{j}")
            for q in range(4):
                nc.sync.dma_start(
                    wt[:, 144 * q : 144 * (q + 1)],
                    wqkv_d[128 * j : 128 * (j + 1), 144 * q : 144 * (q + 1)],
                )
            w_sb.append(wt)
        woutp = []
        for p in range(2):
            wo = wpool.tile([128, D], bf16, tag=f"wo{p}", name=f"wo{p}")
            for q in range(2):
                nc.sync.dma_start(
                    wo[:, 384 * q : 384 * (q + 1)],
                    woutp_d[128 * p : 128 * (p + 1), 384 * q : 384 * (q + 1)],
                )
            woutp.append(wo)

        # PE warmup: throwaway matmuls on an uninitialized tile (output
        # never read) get HAM past the cold window while DMAs are in flight
        wu_t = consts.tile([128, 128], bf16, tag="wu")
        nc.vector.memset(wu_t.bitcast(bf16), 0.25)
        wu_ps = st_ps.tile([128, 512], f32, tag="st", name="st")
        for r in range(14):
            nc.tensor.matmul(wu_ps[:, :128], wu_t, wu_t,
                             start=(r == 0), stop=(r == 13))

        qkvT = [qkvT_pool.tile([128, T], bf16, tag=f"qkvT{i}", name=f"qkvT{i}")
                for i in range(6)]
        # inner stride padded to 128 elems: XBAR transpose dst offsets stay
        # 256B-aligned; ones column at index 64, A@V lhsT reads [:, J, 0:65]
        vaug = [vaug_pool.tile([128, 16, 128], bf16, tag=f"va{h}", name=f"va{h}")
                for h in range(NH)]
        for h in range(NH):
            nc.vector.memset(vaug[h][:, :, DH : DH + 1].bitcast(bf16), 1.0)
        # two [128, T] attention-output tiles: pair1 = [h0; h1], pair2 = [h2; 0]
        ocat = [ocat_pool.tile([128, T], bf16, tag=f"oc{p}", name=f"oc{p}")
                for p in range(2)]
        nc.vector.memset(ocat[1][64:128, :].bitcast(bf16), 0.0)

        xn_tiles = {}

        def emit_ln(g, u):
            i = 4 * g + u
            xt = x_sb[i]
            st = stats.tile([128, 3, 6], f32, tag="bnst", name="bnst")
            for s in range(3):
                nc.vector.bn_stats(st[:, s, :], xt[:, 256 * s : 256 * (s + 1)])
            mv = stats.tile([128, 2], f32, tag="mv", name="mv")
            nc.vector.bn_aggr(mv, st)
            rstd = stats.tile([128, 1], f32, tag="rstd", name="rstd")
            nc.scalar.activation(rstd, mv[:, 1:2], AF.Sqrt, bias=eps_t)
            nc.vector.reciprocal(rstd, rstd)
            xn_t = xn_pool.tile([128, D], bf16, tag=f"xn{i % 8}", name=f"xn{i % 8}")
            nc.vector.tensor_scalar(
                out=xn_t,
                in0=xt,
                scalar1=mv[:, 0:1],
                scalar2=rstd,
                op0=Alu.subtract,
                op1=Alu.mult,
            )
            xn_tiles[i] = xn_t

        def emit_xT(g, j):
            # PE transpose into the shared score psum ring, then evict
            ps = st_ps.tile([128, 512], bf16, tag="st", name="st")
            for u in range(4):
                i = 4 * g + u
                nc.tensor.transpose(
                    ps[:, 128 * u : 128 * (u + 1)],
                    xn_tiles[i][:, 128 * j : 128 * (j + 1)],
                    id_bf,
                )
            dst = xnT[j][:, 512 * g : 512 * (g + 1)]
            if j % 2 == 0:
                nc.scalar.copy(dst, ps)
            else:
                nc.vector.tensor_copy(dst, ps)

        def emit_qkv(g, ci):
            clo, csz = C_CHUNKS[ci]
            pq = mm_ps.tile([128, 512], f32, tag=f"mm{ci % 2}", name=f"mm{ci % 2}")
            for j in range(6):
                nc.tensor.matmul(
                    pq[:csz, :],
                    w_sb[j][:, clo : clo + csz],
                    xnT[j][:, 512 * g : 512 * (g + 1)],
                    start=(j == 0),
                    stop=(j == 5),
                )
            nc.vector.tensor_scalar_add(
                qkvT[ci][:csz, 512 * g : 512 * (g + 1)],
                in0=pq[:csz, :],
                scalar1=bw_sb[:csz, ci : ci + 1],
            )

        def emit_vT(g, h):
            tI, ro = V_LOC[h]
            idsl = id_bf[ro : ro + 64, ro : ro + 64]
            ps = st_ps.tile([128, 512], bf16, tag="st", name="st")
            for u in range(4):
                J = 4 * g + u
                nc.tensor.transpose(
                    ps[:, 64 * u : 64 * (u + 1)],
                    qkvT[tI][ro : ro + 64, 128 * J : 128 * (J + 1)],
                    idsl,
                )
            nc.vector.tensor_copy(
                vaug[h][:, 4 * g : 4 * (g + 1), 0:DH],
                ps[:, 0:256].rearrange("p (u d) -> p u d", u=4),
            )

        def emit_dup(g):
            cols = slice(512 * g, 512 * (g + 1))
            qt, qo = Q_LOC[2]
            kt, ko = K_LOC[2]
            nc.sync.dma_start(qkvT[Q2C[0]][64:128, cols], qkvT[qt][qo : qo + 64, cols])
            nc.sync.dma_start(qkvT[K2C[0]][64:128, cols], qkvT[kt][ko : ko + 64, cols])

        xnT = [xnT_pool.tile([128, T], bf16, tag=f"xnT{j}", name=f"xnT{j}")
               for j in range(6)]

        def grp_thunks(g):
            th = []
            if g > 0:
                # x tiles for this group stream in just-in-time so the DMA
                # queues stay clear for latency-critical transfers
                th.append(lambda: [dma_x(4 * g + u) for u in range(4)])
            for u in range(4):
                th.append(lambda u=u: emit_ln(g, u))
            for j in range(6):
                th.append(lambda j=j: emit_xT(g, j))
            for ci in range(4):
                th.append(lambda ci=ci: emit_qkv(g, ci))
            th.append(lambda: emit_dup(g))
            th.append(lambda: emit_qkv(g, 4))
            for h in range(NH):
                th.append(lambda h=h: emit_vT(g, h))
            return th

        # ---- group 0 up front
        for th in grp_thunks(0):
            th()

        # ---- attention: chunks c = 0..3 of 512 queries; J-pair groups with
        # score row-tile pairing; grp c+1 QKV work interleaved between groups.
        def finalize_act(c, otp, h):
            # 1/den = exp(-ln(den)) on ACT; emitted at group top so it beats
            # the group's six exps into the ACT queue
            recf = rec_pool.tile([128, 512], f32, tag="recf", name="recf")
            nc.scalar.activation(recf[64:65, :], otp[h][64:65, :], AF.Ln)
            rec = rec_pool.tile([128, 512], bf16, tag="rec", name="rec")
            nc.scalar.activation(rec[64:65, :], recf[64:65, :], AF.Exp,
                                 scale=-1.0)
            return rec

        def finalize_one(c, otp, h, rec=None):
            if True:
                if rec is None:
                    rec = finalize_act(c, otp, h)
                # broadcast across 64 partitions via a K=1 matmul (bf16
                # stream); st psum ring reused for the broadcast output
                bcp = st_ps.tile([64, 512], f32, tag="st", name="st")
                nc.tensor.matmul(
                    bcp, ones_t[64:65, :], rec[64:65, :], start=True, stop=True
                )
                rec64 = rec_pool.tile([64, 512], f32, tag="rec64", name="rec64")
                nc.vector.tensor_copy(rec64[:], bcp)
                if debug:
                    dent = tmp_pool.tile([128, 512], f32, tag="dent", name="dent")
                    nc.vector.tensor_copy(dent[64:65, :], otp[h][64:65, :])
                    nc.sync.dma_start(dbg_den_d[c, h], dent[64:65, :])
                    nc.sync.dma_start(dbg_rec_d[c, h], rec64[0:1, :])
                if h == 0:
                    nc.vector.tensor_mul(
                        ocat[0][0:64, 512 * c : 512 * (c + 1)],
                        otp[h][0:DH, :],
                        rec64[:],
                    )
                elif h == 2:
                    nc.vector.tensor_mul(
                        ocat[1][0:64, 512 * c : 512 * (c + 1)],
                        otp[h][0:DH, :],
                        rec64[:],
                    )
                else:
                    tmp = tmp_pool.tile([64, 512], bf16, tag="tmp1", name="tmp1")
                    nc.vector.tensor_mul(tmp[:], otp[h][0:DH, :], rec64[:])
                    nc.sync.dma_start(
                        ocat[0][64:128, 512 * c : 512 * (c + 1)], tmp[:]
                    )

        def finalize(c, otp):
            for h in range(NH):
                finalize_one(c, otp, h)

        def emit_oproj(t):
            ot_sb = io_pool.tile([128, D], f32, tag="osb", name="osb")
            for eh in range(2):
                opp = mm_ps.tile([128, 512], f32, tag=f"mm{eh}", name=f"mm{eh}")
                nc.tensor.matmul(
                    opp[:, :384],
                    ocat[0][:, 128 * t : 128 * (t + 1)],
                    woutp[0][:, 384 * eh : 384 * (eh + 1)],
                    start=True,
                    stop=False,
                )
                nc.tensor.matmul(
                    opp[:, :384],
                    ocat[1][:, 128 * t : 128 * (t + 1)],
                    woutp[1][:, 384 * eh : 384 * (eh + 1)],
                    start=False,
                    stop=True,
                )
                nc.vector.tensor_copy(
                    ot_sb[:, 384 * eh : 384 * (eh + 1)], opp[:, :384]
                )
            nc.sync.dma_start(out_d[128 * t : 128 * (t + 1), :], ot_sb)

        deferred = None
        for c in range(4):
            otp = [ot_ps.tile([DH + 1, 512], f32, tag=f"ot{h}", name=f"ot{h}")
                   for h in range(NH)]
            nJ = 4 * c + 4
            ngroups = nJ // 2
            thunks = grp_thunks(c + 1) if c < 3 else []
            tpop = 0
            pending = []
            oproj_todo = []

            def emit_avs(avs):
                for (h, J, s0, n, pt) in avs:
                    nc.tensor.matmul(
                        otp[h][:, s0:512],
                        vaug[h][:, J, 0 : DH + 1],
                        pt[:, :n],
                        start=(J == 0),
                        stop=(J == nJ - 1),
                    )

            for gi in range(ngroups):
                J0, J1 = 2 * gi, 2 * gi + 1
                frec = None
                if deferred is not None and gi < 3:
                    frec = finalize_act(deferred[0], deferred[1], gi)
                prm = []  # (h, J, use_hi_copy)
                prm.append((0, J0, False))
                prm.append((1, J0, False))
                prm.append((2, J0, False))
                prm.append((2, J1, True))
                prm.append((0, J1, False))
                prm.append((1, J1, False))
                avs = []
                # previous group's AVs emitted in halves BETWEEN score pairs:
                # the PE queue is in-order, so this gives each score's exp
                # ~2us of PE cover before its psum slot is needed again
                prev_avs = pending.pop(0) if pending else []
                for pi, (h, J, hic) in enumerate(prm):
                    s0 = max(0, 128 * J - 512 * c)
                    n = 512 - s0
                    q0 = 512 * c + s0
                    if hic:
                        qt, qo = Q2C
                        kt, ko = K2C
                    else:
                        qt, qo = Q_LOC[h]
                        kt, ko = K_LOC[h]
                    stp = st_ps.tile([128, 512], f32, tag="st", name="st")
                    nc.tensor.matmul(
                        stp[:, :n],
                        qkvT[kt][ko : ko + 64, 128 * J : 128 * (J + 1)],
                        qkvT[qt][qo : qo + 64, q0 : q0 + n],
                        start=True,
                        stop=True,
                    )
                    pt = pt_pool.tile([128, 512], bf16, tag=f"pt{len(avs) % 6}",
                                      name="pt")
                    nc.scalar.activation(
                        pt[:, :n], stp[:, :n], AF.Exp,
                        scale=float(DH) ** -0.5,
                    )
                    if J >= 4 * c:
                        nc.vector.memset(pt[64:128, 0:64].bitcast(bf16), 0.0)
                    avs.append((h, J, s0, n, pt))
                    if pi == 1:
                        emit_avs(prev_avs[0:3])
                    elif pi == 3:
                        emit_avs(prev_avs[3:6])
                # re-sort AVs into J order (J0's three heads then J1's)
                avs.sort(key=lambda a: (a[1], a[0]))
                pending.append(avs)
                if deferred is not None and gi < 3:
                    finalize_one(deferred[0], deferred[1], gi, rec=frec)
                    if gi == 2 or gi == ngroups - 1:
                        oproj_todo = list(
                            range(4 * deferred[0], 4 * deferred[0] + 4)
                        )
                        deferred = None
                # interleave one deferred out-proj token tile per group
                if oproj_todo:
                    emit_oproj(oproj_todo.pop(0))
                # interleave next group's LN/transpose/QKV thunks
                want = math.ceil(len(thunks) * (gi + 1) / ngroups)
                while tpop < want:
                    thunks[tpop]()
                    tpop += 1
            while pending:
                emit_avs(pending.pop(0))
            while oproj_todo:
                emit_oproj(oproj_todo.pop(0))
            deferred = (c, otp)
        finalize(*deferred)
        for t in range(12, 16):
            emit_oproj(t)

        if debug:
            for i in range(6):
                nc.sync.dma_start(dbg_qkvT_d[i], qkvT[i][:])
            for h in range(NH):
                nc.sync.dma_start(dbg_vaug_d[h], vaug[h][:])
            for p in range(2):
                nc.sync.dma_start(dbg_ocat_d[p], ocat[p][:])

    return nc


def shard_inputs(x, gamma, beta, w_qkv, w_out, b_out):
    """Full inputs -> list of 8 per-core input dicts (host-side weight prep)."""
    import ml_dtypes

    bfloat16 = ml_dtypes.bfloat16
    x = np.ascontiguousarray(np.asarray(x, dtype=np.float32))
    gamma = np.asarray(gamma, dtype=np.float32)
    beta = np.asarray(beta, dtype=np.float32)
    w_qkv = np.asarray(w_qkv, dtype=np.float32)
    w_out = np.asarray(w_out, dtype=np.float32)
    in_maps = []
    for g in range(NCORES):
        b = g // 4
        hg = g % 4
        heads = [3 * hg + h for h in range(NH)]
        segs = []
        for kind, h in SEG_ORDER:
            hh = heads[h]
            base = {"q": 0, "k": D, "v": 2 * D}[kind]
            segs.append(w_qkv[:, base + 64 * hh : base + 64 * (hh + 1)])
        wqkv_g = np.ascontiguousarray(np.concatenate(segs, axis=1))
        bw_g = beta @ wqkv_g  # [CH] f32, raw weights
        bw_pack = np.zeros(640, dtype=np.float32)
        bw_pack[:CH] = bw_g
        wqkv_bf = (wqkv_g * gamma[:, None]).astype(bfloat16)
        wout_g = w_out[64 * heads[0] : 64 * (heads[-1] + 1), :]
        woutp = np.zeros((256, D), dtype=np.float32)
        woutp[0:128] = wout_g[0:128]
        woutp[128:192] = wout_g[128:192]
        in_maps.append(
            {
                "x": x[b],
                "wqkv": np.ascontiguousarray(wqkv_bf),
                "woutp": woutp.astype(bfloat16),
                "bw": bw_pack,
                "ident": np.eye(128, dtype=np.float32).astype(bfloat16),
            }
        )
    return in_maps


def kernel(x, gamma, beta, w_qkv, w_out, b_out):
    _install_axon_hooks_shim()
    from concourse import bass_utils

    if "nc" not in _CACHE:
        _CACHE["nc"] = build_nc()
    nc = _CACHE["nc"]

    in_maps = shard_inputs(x, gamma, beta, w_qkv, w_out, b_out)
    trace = bool(int(os.environ.get("KERNEL_TRACE", "0")))
    kwargs = {}
    if trace:
        kwargs["trace"] = True
        tmpdir = os.environ.get("KERNEL_TRACE_DIR")
        if tmpdir:
            kwargs["tmpdir"] = tmpdir
        # artifact upload needs external storage; keep the trace local
        bass_utils.upload_artifacts = lambda d: d
    res = bass_utils.run_bass_kernel_spmd(
        nc, in_maps, list(range(NCORES)), **kwargs
    )
    _CACHE["last_exec_time_ns"] = res.exec_time_ns

    b_out = np.asarray(b_out, dtype=np.float32)
    out = np.empty((B, T, D), dtype=np.float32)
    for b in range(B):
        acc = res.results[4 * b]["out"].astype(np.float32)
        for hg in range(1, 4):
            acc = acc + res.results[4 * b + hg]["out"]
        out[b] = acc + b_out[None, :]
    return out


# revision 29
# speedup vs baseline: 1.0098x; 1.0098x over previous
"""Trainium2 Bass kernel for nn_Attention_55233279426826 (block-causal attention).

Reference computation (per batch b):
    xn = LayerNorm(x[b]) * gamma + beta
    q,k,v = split(xn @ w_qkv), 12 heads x 64
    attn  = softmax(block-causal-masked(q k^T / 8))
    out[b] = (attn v) @ w_out + b_out

Sharding (8 cores): batch (2) x head-group (4, 3 heads each).  Each core gets
its batch's x, the w_qkv columns and w_out rows of its 3 heads, and produces a
partial [2048, 768] output.  Host sums the 4 head-group partials per batch and
adds b_out.  gamma is folded into w_qkv on host; beta@w_qkv is a host-computed
per-channel bias added at QKV psum eviction.  Weights ship as bf16.

Per-core device program:
  1. LayerNorm stats in [token, dim] layout (bn_stats/bn_aggr), apply
     (x - mu) * rstd on DVE -> bf16 xn.
  2. xn -> xnT [768, 2048] via XBAR DMA transposes (off the PE).
  3. qkvT [576, 2048] = w_qkv^T @ xnT on PE; head strips laid out so the two
     heads of a score pair sit on opposite partition halves (auto row-tiling:
     two 64x128 score matmuls run concurrently in the PE array).  Head 2's
     q/k strips are duplicated to the hi partition half via SBUF DMA so odd-J
     score matmuls pair with even-J ones.
  4. v re-transposed to [keys, 64] via XBAR DMA, augmented with a ones column
     (index 64) so A@V also produces softmax denominators in psum row 64.
  5. Scores S_T[j, q] per 128-key block J / 512-query chunk c, exp on ACT
     (bounded scores, no max pass), masked corners zeroed, A@V into psum.
     1/den via DVE reciprocal + GPSIMD partition_broadcast (no PE involved),
     divided out during psum->SBUF eviction.
  6. Attention outputs packed into two [128, T] tiles ([h0;h1] and [h2;0]) so
     the out-projection runs as K=128 full-array matmuls; streamed to DRAM.
  7. QKV/LN/transpose work for token group g+1 is interleaved into attention
     chunk c=g so PE, ACT, DVE and DMA all stay busy.
"""

import contextlib
import ctypes
import math
import os
import sys
import types

import numpy as np

B = 2
T = 2048
D = 768
NPATCH = 64
HEADS = 12
DH = 64
NH = 3          # heads per core
CH = 3 * NH * DH  # 576 qkv channels per core
LN_EPS = 1e-5
NCORES = 8

_CACHE = {}


def _install_axon_hooks_shim():
    """This image's antenv lacks axon_hooks; synthesize it so that
    run_bass_kernel_spmd(trace=True) finds the NTFF profile hook instead of
    crashing on import.  Safe no-op if profiling symbols are unavailable."""
    if "antenv.axon_hooks" in sys.modules:
        return
    mod = types.ModuleType("antenv.axon_hooks")
    _hook = [None]
    mod.set_axon_ntff_profile_hook = lambda h: _hook.__setitem__(0, h)
    mod.get_axon_ntff_profile_hook = lambda: _hook[0]
    sys.modules["antenv.axon_hooks"] = mod
    try:
        lib = ctypes.CDLL("/opt/axon/libaxon_pjrt.so")
        if not hasattr(lib, "axon_start_nrt_profile"):
            return
        lib.axon_start_nrt_profile.argtypes = [
            ctypes.POINTER(ctypes.c_int64),
            ctypes.c_size_t,
        ]
        lib.axon_start_nrt_profile.restype = ctypes.c_int64
        lib.axon_stop_nrt_profile.argtypes = [ctypes.c_char_p]
        lib.axon_stop_nrt_profile.restype = ctypes.c_int64

        @contextlib.contextmanager
        def _hook_cm(output_dir, device_ids):
            import jax

            jax.devices()
            if device_ids:
                ids = (ctypes.c_int64 * len(device_ids))(*device_ids)
                rc = lib.axon_start_nrt_profile(ids, len(device_ids))
            else:
                rc = lib.axon_start_nrt_profile(None, 0)
            if rc != 0:
                raise RuntimeError(f"axon_start_nrt_profile rc={rc}")
            try:
                yield
            finally:
                n = lib.axon_stop_nrt_profile(str(output_dir).encode())
                print(f"profile: {n} file(s) -> {output_dir}", file=sys.stderr)

        mod.set_axon_ntff_profile_hook(_hook_cm)
    except OSError:
        pass


def _install_drain_split():
    """The walrus build in this container accepts only ONE sync wait per
    CTRL(drain) instruction; Tile's tail drain carries several.  Split the
    waits across a chain of drains."""
    import bass_rust
    import concourse.tile as tile
    from concourse.vector_clock import ScopedClock

    if getattr(tile.TileContext, "_drain_split_installed", False):
        return

    def _drain_and_barrier(self, tick_clock, wait_clock):
        nc = self.nc
        drain_inst = nc.sync.drain()
        wait_clock.add_sem_waits(
            drain_inst.ins, ScopedClock({None: tick_clock.global_clock})
        )
        si = drain_inst.ins.sync_info
        if si is not None:
            waits = list(si.on_wait)
            if len(waits) > 1:
                si.on_wait = waits[:1]
                for w in waits[1:]:
                    extra = nc.sync.drain()
                    extra.ins.sync_info = bass_rust.SyncInfo(
                        on_wait=[w], on_update=[]
                    )
        nc.all_engine_barrier()
        popped = nc._tile_sem_poison_stack.pop()
        assert popped is self._sem_poison
        nc.clear_and_free_semaphores(list(self.sems.allocated().values()))
        nc.all_engine_barrier()

    tile.TileContext._drain_and_barrier = _drain_and_barrier

    # Generic pass: walrus here allows 1 sync wait per instruction; move
    # extra waits onto nofuse NOPs inserted just before, on the same engine.
    from concourse import mybir

    orig_lower = tile.TileContext._lower_ordered_insts

    def _lower_split(self, ordered):
        for insts in ordered.values():
            new = []
            for inst in insts:
                si = getattr(inst, "sync_info", None)
                eng = getattr(inst, "engine", None)
                if si is not None and eng is not None:
                    waits = list(si.on_wait)
                    if len(waits) > 1:
                        movable = [w for w in waits
                                   if getattr(w, "sync_type", "") == "semaphore"]
                        keep = [w for w in waits if w not in movable]
                        if not keep:
                            keep = [movable.pop()]
                        for k, w in enumerate(movable):
                            nop = mybir.InstNoOp(
                                name=f"{inst.name}-wsplit{k}",
                                sync_info=mybir.SyncInfo(
                                    on_wait=[w], on_update=[]
                                ),
                                bass_nofuse=True,
                                engine=eng,
                            )
                            new.append(nop)
                        inst.sync_info = mybir.SyncInfo(
                            on_wait=keep, on_update=list(si.on_update)
                        )
                new.append(inst)
            insts[:] = new
        return orig_lower(self, ordered)

    tile.TileContext._lower_ordered_insts = _lower_split
    tile.TileContext._drain_split_installed = True


# qkvT row layout over six [128, T] tiles (64-row strips):
# t0 = [q0; q1], t1 = [k0; k1], t2 = [q2; v0], t3 = [k2; v1],
# t4 = [v2; k2copy], t5 = [--; q2copy]
# q and k of the same head share a partition offset (matmul operands must have
# equal base partitions); the head-2 hi copies land via SBUF->SBUF DMA.
Q_LOC = [(0, 0), (0, 64), (2, 0)]
K_LOC = [(1, 0), (1, 64), (3, 0)]
V_LOC = [(2, 64), (3, 64), (4, 0)]
Q2C = (5, 64)
K2C = (4, 64)
# host column order of the permuted per-core w_qkv (64-col segments)
SEG_ORDER = [("q", 0), ("q", 1), ("k", 0), ("k", 1), ("q", 2), ("v", 0),
             ("k", 2), ("v", 1), ("v", 2)]

C_CHUNKS = [(0, 128), (128, 128), (256, 128), (384, 128), (512, 64)]


def build_nc():
    import concourse.bass as bass
    import concourse.tile as tile
    from concourse import mybir

    _install_drain_split()

    f32 = mybir.dt.float32
    bf16 = mybir.dt.bfloat16
    AF = mybir.ActivationFunctionType
    Alu = mybir.AluOpType

    debug = bool(int(os.environ.get("KERNEL_DEBUG", "0")))
    nc = bass.Bass()
    x_d = nc.dram_tensor("x", [T, D], f32, kind="ExternalInput")
    wqkv_d = nc.dram_tensor("wqkv", [D, CH], bf16, kind="ExternalInput")
    woutp_d = nc.dram_tensor("woutp", [256, D], bf16, kind="ExternalInput")
    bw_d = nc.dram_tensor("bw", [640], f32, kind="ExternalInput")
    ident_d = nc.dram_tensor("ident", [128, 128], bf16, kind="ExternalInput")
    out_d = nc.dram_tensor("out", [T, D], f32, kind="ExternalOutput")
    if debug:
        dbg_qkvT_d = nc.dram_tensor("dbg_qkvT", [6, 128, T], bf16,
                                    kind="ExternalOutput")
        dbg_vaug_d = nc.dram_tensor("dbg_vaug", [NH, 128, 16, 128], bf16,
                                    kind="ExternalOutput")
        dbg_ocat_d = nc.dram_tensor("dbg_ocat", [2, 128, T], bf16,
                                    kind="ExternalOutput")
        dbg_den_d = nc.dram_tensor("dbg_den", [4, NH, 1, 512], f32,
                                   kind="ExternalOutput")
        dbg_rec_d = nc.dram_tensor("dbg_rec", [4, NH, 1, 512], f32,
                                   kind="ExternalOutput")

    with contextlib.ExitStack() as ctx:
        ctx.enter_context(
            nc.allow_low_precision(reason="bf16 PE inputs are intentional")
        )
        tc = ctx.enter_context(tile.TileContext(nc))
        consts = ctx.enter_context(tc.tile_pool(name="consts", bufs=1))
        wpool = ctx.enter_context(tc.tile_pool(name="w", bufs=1))
        qkvT_pool = ctx.enter_context(tc.tile_pool(name="qkvT", bufs=1))
        vaug_pool = ctx.enter_context(tc.tile_pool(name="vaug", bufs=1))
        ocat_pool = ctx.enter_context(tc.tile_pool(name="ocat", bufs=1))
        xpool = ctx.enter_context(tc.tile_pool(name="xin", bufs=1))
        xn_pool = ctx.enter_context(tc.tile_pool(name="xn", bufs=1))
        xnT_pool = ctx.enter_context(tc.tile_pool(name="xnT", bufs=1))
        io_pool = ctx.enter_context(tc.tile_pool(name="io", bufs=3))
        stats = ctx.enter_context(tc.tile_pool(name="stats", bufs=4))
        # bufs=2: group g+1's exp (emitted before group g's deferred A@V)
        # must not alias the pt tiles that A@V still reads
        pt_pool = ctx.enter_context(tc.tile_pool(name="pt", bufs=2))
        rec_pool = ctx.enter_context(tc.tile_pool(name="rec", bufs=2))
        tmp_pool = ctx.enter_context(tc.tile_pool(name="tmp", bufs=2))
        # PSUM banks: 2 (qkv/out-proj shared, tags mm0/mm1) + 3 (scores)
        # + 3 (attn out, tags ot0-2) = 8
        mm_ps = ctx.enter_context(tc.tile_pool(name="mm_ps", bufs=1, space="PSUM"))
        st_ps = ctx.enter_context(tc.tile_pool(name="st_ps", bufs=3, space="PSUM"))
        ot_ps = ctx.enter_context(tc.tile_pool(name="ot_ps", bufs=1, space="PSUM"))

        id_bf = consts.tile([128, 128], bf16, tag="idbf")
        nc.sync.dma_start(id_bf, ident_d[:, :])
        eps_t = consts.tile([128, 1], f32, tag="eps")
        nc.vector.memset(eps_t, LN_EPS)
        ones_t = consts.tile([128, DH], bf16, tag="ones")
        nc.vector.memset(ones_t.bitcast(bf16), 1.0)

        # DMA order: group-0 x tiles first (LN gates the pipeline), then
        # weights (QKV needs them by ~15us), then the rest of x.  Everything
        # split 4-way so no tile's arrival is bound to one ~20GB/s queue.
        x_sb = [xpool.tile([128, D], f32, tag=f"x{i}", name=f"xx{i}")
                for i in range(16)]

        def dma_x(i):
            for q in range(4):
                nc.sync.dma_start(
                    x_sb[i][:, 192 * q : 192 * (q + 1)],
                    x_d[128 * i : 128 * (i + 1), 192 * q : 192 * (q + 1)],
                )

        for i in range(4):
            dma_x(i)
        bw_sb = consts.tile([128, 5], f32, tag="bw")
        nc.sync.dma_start(bw_sb, bw_d[:].rearrange("(a p) -> p a", p=128))
        w_sb = []
        for j in range(6):
            wt = wpool.tile([128, CH], bf16, tag=f"w{j}", name=f"w{j}")
            for q in range(4):
                nc.sync.dma_start(
                    wt[:, 144 * q : 144 * (q + 1)],
                    wqkv_d[128 * j : 128 * (j + 1), 144 * q : 144 * (q + 1)],
                )
            w_sb.append(wt)
        woutp = []
        for p in range(2):
            wo = wpool.tile([128, D], bf16, tag=f"wo{p}", name=f"wo{p}")
            for q in range(2):
                nc.sync.dma_start(
                    wo[:, 384 * q : 384 * (q + 1)],
                    woutp_d[128 * p : 128 * (p + 1), 384 * q : 384 * (q + 1)],
                )
            woutp.append(wo)

        # PE warmup: throwaway matmuls on an uninitialized tile (output
        # never read) get HAM past the cold window while DMAs are in flight
        wu_t = consts.tile([128, 128], bf16, tag="wu")
        nc.vector.memset(wu_t.bitcast(bf16), 0.25)
        wu_ps = st_ps.tile([128, 512], f32, tag="st", name="st")
        for r in range(14):
            nc.tensor.matmul(wu_ps[:, :128], wu_t, wu_t,
                             start=(r == 0), stop=(r == 13))

        qkvT = [qkvT_pool.tile([128, T], bf16, tag=f"qkvT{i}", name=f"qkvT{i}")
                for i in range(6)]
        # inner stride padded to 128 elems: XBAR transpose dst offsets stay
        # 256B-aligned; ones column at index 64, A@V lhsT reads [:, J, 0:65]
        vaug = [vaug_pool.tile([128, 16, 128], bf16, tag=f"va{h}", name=f"va{h}")
                for h in range(NH)]
        for h in range(NH):
            nc.vector.memset(vaug[h][:, :, DH : DH + 1].bitcast(bf16), 1.0)
        # two [128, T] attention-output tiles: pair1 = [h0; h1], pair2 = [h2; 0]
        ocat = [ocat_pool.tile([128, T], bf16, tag=f"oc{p}", name=f"oc{p}")
                for p in range(2)]
        nc.vector.memset(ocat[1][64:128, :].bitcast(bf16), 0.0)

        xn_tiles = {}

        def emit_ln(g, u):
            i = 4 * g + u
            xt = x_sb[i]
            st = stats.tile([128, 3, 6], f32, tag="bnst", name="bnst")
            for s in range(3):
                nc.vector.bn_stats(st[:, s, :], xt[:, 256 * s : 256 * (s + 1)])
            mv = stats.tile([128, 2], f32, tag="mv", name="mv")
            nc.vector.bn_aggr(mv, st)
            rstd = stats.tile([128, 1], f32, tag="rstd", name="rstd")
            nc.scalar.activation(rstd, mv[:, 1:2], AF.Sqrt, bias=eps_t)
            nc.vector.reciprocal(rstd, rstd)
            xn_t = xn_pool.tile([128, D], bf16, tag=f"xn{i % 8}", name=f"xn{i % 8}")
            nc.vector.tensor_scalar(
                out=xn_t,
                in0=xt,
                scalar1=mv[:, 0:1],
                scalar2=rstd,
                op0=Alu.subtract,
                op1=Alu.mult,
            )
            xn_tiles[i] = xn_t

        def emit_xT(g, j):
            # PE transpose into the shared score psum ring, then evict
            ps = st_ps.tile([128, 512], bf16, tag="st", name="st")
            for u in range(4):
                i = 4 * g + u
                nc.tensor.transpose(
                    ps[:, 128 * u : 128 * (u + 1)],
                    xn_tiles[i][:, 128 * j : 128 * (j + 1)],
                    id_bf,
                )
            dst = xnT[j][:, 512 * g : 512 * (g + 1)]
            if g == 0 or j % 2 == 0:
                nc.scalar.copy(dst, ps)
            else:
                nc.vector.tensor_copy(dst, ps)

        def emit_qkv(g, ci):
            clo, csz = C_CHUNKS[ci]
            pq = mm_ps.tile([128, 512], f32, tag=f"mm{ci % 2}", name=f"mm{ci % 2}")
            halves = ((0, 256), (256, 256)) if g == 0 else ((0, 512),)
            for hlo, hsz in halves:
                for j in range(6):
                    nc.tensor.matmul(
                        pq[:csz, hlo : hlo + hsz],
                        w_sb[j][:, clo : clo + csz],
                        xnT[j][:, 512 * g + hlo : 512 * g + hlo + hsz],
                        start=(j == 0),
                        stop=(j == 5),
                    )
            nc.vector.tensor_scalar_add(
                qkvT[ci][:csz, 512 * g : 512 * (g + 1)],
                in0=pq[:csz, :],
                scalar1=bw_sb[:csz, ci : ci + 1],
            )

        def emit_vT(g, h):
            tI, ro = V_LOC[h]
            idsl = id_bf[ro : ro + 64, ro : ro + 64]
            ps = st_ps.tile([128, 512], bf16, tag="st", name="st")
            for u in range(4):
                J = 4 * g + u
                nc.tensor.transpose(
                    ps[:, 64 * u : 64 * (u + 1)],
                    qkvT[tI][ro : ro + 64, 128 * J : 128 * (J + 1)],
                    idsl,
                )
            nc.vector.tensor_copy(
                vaug[h][:, 4 * g : 4 * (g + 1), 0:DH],
                ps[:, 0:256].rearrange("p (u d) -> p u d", u=4),
            )

        def emit_dup(g):
            cols = slice(512 * g, 512 * (g + 1))
            qt, qo = Q_LOC[2]
            kt, ko = K_LOC[2]
            nc.sync.dma_start(qkvT[Q2C[0]][64:128, cols], qkvT[qt][qo : qo + 64, cols])
            nc.sync.dma_start(qkvT[K2C[0]][64:128, cols], qkvT[kt][ko : ko + 64, cols])

        xnT = [xnT_pool.tile([128, T], bf16, tag=f"xnT{j}", name=f"xnT{j}")
               for j in range(6)]

        def grp_thunks(g):
            th = []
            if g > 0:
                # x tiles for this group stream in just-in-time so the DMA
                # queues stay clear for latency-critical transfers
                th.append(lambda: [dma_x(4 * g + u) for u in range(4)])
            for u in range(4):
                th.append(lambda u=u: emit_ln(g, u))
            for j in range(6):
                th.append(lambda j=j: emit_xT(g, j))
            for ci in range(4):
                th.append(lambda ci=ci: emit_qkv(g, ci))
            th.append(lambda: emit_dup(g))
            th.append(lambda: emit_qkv(g, 4))
            for h in range(NH):
                th.append(lambda h=h: emit_vT(g, h))
            return th

        # ---- group 0 up front
        for th in grp_thunks(0):
            th()

        # ---- attention: chunks c = 0..3 of 512 queries; J-pair groups with
        # score row-tile pairing; grp c+1 QKV work interleaved between groups.
        def finalize_act(c, otp, h):
            # 1/den = exp(-ln(den)) on ACT; emitted at group top so it beats
            # the group's six exps into the ACT queue
            recf = rec_pool.tile([128, 512], f32, tag="recf", name="recf")
            nc.scalar.activation(recf[64:65, :], otp[h][64:65, :], AF.Ln)
            rec = rec_pool.tile([128, 512], bf16, tag="rec", name="rec")
            nc.scalar.activation(rec[64:65, :], recf[64:65, :], AF.Exp,
                                 scale=-1.0)
            return rec

        def finalize_one(c, otp, h, rec=None):
            if True:
                if rec is None:
                    rec = finalize_act(c, otp, h)
                # broadcast across 64 partitions via a K=1 matmul (bf16
                # stream); st psum ring reused for the broadcast output
                bcp = st_ps.tile([64, 512], f32, tag="st", name="st")
                nc.tensor.matmul(
                    bcp, ones_t[64:65, :], rec[64:65, :], start=True, stop=True
                )
                rec64 = rec_pool.tile([64, 512], f32, tag="rec64", name="rec64")
                nc.vector.tensor_copy(rec64[:], bcp)
                if debug:
                    dent = tmp_pool.tile([128, 512], f32, tag="dent", name="dent")
                    nc.vector.tensor_copy(dent[64:65, :], otp[h][64:65, :])
                    nc.sync.dma_start(dbg_den_d[c, h], dent[64:65, :])
                    nc.sync.dma_start(dbg_rec_d[c, h], rec64[0:1, :])
                if h == 0:
                    nc.vector.tensor_mul(
                        ocat[0][0:64, 512 * c : 512 * (c + 1)],
                        otp[h][0:DH, :],
                        rec64[:],
                    )
                elif h == 2:
                    nc.vector.tensor_mul(
                        ocat[1][0:64, 512 * c : 512 * (c + 1)],
                        otp[h][0:DH, :],
                        rec64[:],
                    )
                else:
                    tmp = tmp_pool.tile([64, 512], bf16, tag="tmp1", name="tmp1")
                    nc.vector.tensor_mul(tmp[:], otp[h][0:DH, :], rec64[:])
                    nc.sync.dma_start(
                        ocat[0][64:128, 512 * c : 512 * (c + 1)], tmp[:]
                    )

        def finalize(c, otp):
            for h in range(NH):
                finalize_one(c, otp, h)

        def emit_oproj(t):
            ot_sb = io_pool.tile([128, D], f32, tag="osb", name="osb")
            for eh in range(2):
                opp = mm_ps.tile([128, 512], f32, tag=f"mm{eh}", name=f"mm{eh}")
                nc.tensor.matmul(
                    opp[:, :384],
                    ocat[0][:, 128 * t : 128 * (t + 1)],
                    woutp[0][:, 384 * eh : 384 * (eh + 1)],
                    start=True,
                    stop=False,
                )
                nc.tensor.matmul(
                    opp[:, :384],
                    ocat[1][:, 128 * t : 128 * (t + 1)],
                    woutp[1][:, 384 * eh : 384 * (eh + 1)],
                    start=False,
                    stop=True,
                )
                nc.vector.tensor_copy(
                    ot_sb[:, 384 * eh : 384 * (eh + 1)], opp[:, :384]
                )
            nc.sync.dma_start(out_d[128 * t : 128 * (t + 1), :], ot_sb)

        deferred = None
        for c in range(4):
            otp = [ot_ps.tile([DH + 1, 512], f32, tag=f"ot{h}", name=f"ot{h}")
                   for h in range(NH)]
            nJ = 4 * c + 4
            ngroups = nJ // 2
            thunks = grp_thunks(c + 1) if c < 3 else []
            tpop = 0
            pending = []
            oproj_todo = []

            def emit_avs(avs):
                for (h, J, s0, n, pt) in avs:
                    nc.tensor.matmul(
                        otp[h][:, s0:512],
                        vaug[h][:, J, 0 : DH + 1],
                        pt[:, :n],
                        start=(J == 0),
                        stop=(J == nJ - 1),
                    )

            for gi in range(ngroups):
                J0, J1 = 2 * gi, 2 * gi + 1
                frec = None
                if deferred is not None and gi < 3:
                    frec = finalize_act(deferred[0], deferred[1], gi)
                prm = []  # (h, J, use_hi_copy)
                prm.append((0, J0, False))
                prm.append((1, J0, False))
                prm.append((2, J0, False))
                prm.append((2, J1, True))
                prm.append((0, J1, False))
                prm.append((1, J1, False))
                avs = []
                for pi, (h, J, hic) in enumerate(prm):
                    s0 = max(0, 128 * J - 512 * c)
                    n = 512 - s0
                    q0 = 512 * c + s0
                    if hic:
                        qt, qo = Q2C
                        kt, ko = K2C
                    else:
                        qt, qo = Q_LOC[h]
                        kt, ko = K_LOC[h]
                    stp = st_ps.tile([128, 512], f32, tag="st", name="st")
                    nc.tensor.matmul(
                        stp[:, :n],
                        qkvT[kt][ko : ko + 64, 128 * J : 128 * (J + 1)],
                        qkvT[qt][qo : qo + 64, q0 : q0 + n],
                        start=True,
                        stop=True,
                    )
                    pt = pt_pool.tile([128, 512], bf16, tag=f"pt{len(avs) % 6}",
                                      name="pt")
                    nc.scalar.activation(
                        pt[:, :n], stp[:, :n], AF.Exp,
                        scale=float(DH) ** -0.5,
                    )
                    if J >= 4 * c:
                        nc.vector.memset(pt[64:128, 0:64].bitcast(bf16), 0.0)
                    avs.append((h, J, s0, n, pt))
                # re-sort AVs into J order (J0's three heads then J1's)
                avs.sort(key=lambda a: (a[1], a[0]))
                pending.append(avs)
                if len(pending) > 1:
                    emit_avs(pending.pop(0))
                if deferred is not None and gi < 3:
                    finalize_one(deferred[0], deferred[1], gi, rec=frec)
                    if gi == 2 or gi == ngroups - 1:
                        oproj_todo = list(
                            range(4 * deferred[0], 4 * deferred[0] + 4)
                        )
                        deferred = None
                # interleave one deferred out-proj token tile per group
                if oproj_todo:
                    emit_oproj(oproj_todo.pop(0))
                # interleave next group's LN/transpose/QKV thunks
                want = math.ceil(len(thunks) * (gi + 1) / ngroups)
                while tpop < want:
                    thunks[tpop]()
                    tpop += 1
            while pending:
                emit_avs(pending.pop(0))
            while oproj_todo:
                emit_oproj(oproj_todo.pop(0))
            deferred = (c, otp)
        finalize(*deferred)
        for t in range(12, 16):
            emit_oproj(t)

        if debug:
            for i in range(6):
                nc.sync.dma_start(dbg_qkvT_d[i], qkvT[i][:])
            for h in range(NH):
                nc.sync.dma_start(dbg_vaug_d[h], vaug[h][:])
            for p in range(2):
                nc.sync.dma_start(dbg_ocat_d[p], ocat[p][:])

    return nc


def shard_inputs(x, gamma, beta, w_qkv, w_out, b_out):
    """Full inputs -> list of 8 per-core input dicts (host-side weight prep)."""
    import ml_dtypes

    bfloat16 = ml_dtypes.bfloat16
    x = np.ascontiguousarray(np.asarray(x, dtype=np.float32))
    gamma = np.asarray(gamma, dtype=np.float32)
    beta = np.asarray(beta, dtype=np.float32)
    w_qkv = np.asarray(w_qkv, dtype=np.float32)
    w_out = np.asarray(w_out, dtype=np.float32)
    in_maps = []
    for g in range(NCORES):
        b = g // 4
        hg = g % 4
        heads = [3 * hg + h for h in range(NH)]
        segs = []
        for kind, h in SEG_ORDER:
            hh = heads[h]
            base = {"q": 0, "k": D, "v": 2 * D}[kind]
            segs.append(w_qkv[:, base + 64 * hh : base + 64 * (hh + 1)])
        wqkv_g = np.ascontiguousarray(np.concatenate(segs, axis=1))
        bw_g = beta @ wqkv_g  # [CH] f32, raw weights
        bw_pack = np.zeros(640, dtype=np.float32)
        bw_pack[:CH] = bw_g
        wqkv_bf = (wqkv_g * gamma[:, None]).astype(bfloat16)
        wout_g = w_out[64 * heads[0] : 64 * (heads[-1] + 1), :]
        woutp = np.zeros((256, D), dtype=np.float32)
        woutp[0:128] = wout_g[0:128]
        woutp[128:192] = wout_g[128:192]
        in_maps.append(
            {
                "x": x[b],
                "wqkv": np.ascontiguousarray(wqkv_bf),
                "woutp": woutp.astype(bfloat16),
                "bw": bw_pack,
                "ident": np.eye(128, dtype=np.float32).astype(bfloat16),
            }
        )
    return in_maps


def kernel(x, gamma, beta, w_qkv, w_out, b_out):
    _install_axon_hooks_shim()
    from concourse import bass_utils

    if "nc" not in _CACHE:
        _CACHE["nc"] = build_nc()
    nc = _CACHE["nc"]

    in_maps = shard_inputs(x, gamma, beta, w_qkv, w_out, b_out)
    trace = bool(int(os.environ.get("KERNEL_TRACE", "0")))
    kwargs = {}
    if trace:
        kwargs["trace"] = True
        tmpdir = os.environ.get("KERNEL_TRACE_DIR")
        if tmpdir:
            kwargs["tmpdir"] = tmpdir
        # artifact upload needs external storage; keep the trace local
        bass_utils.upload_artifacts = lambda d: d
    res = bass_utils.run_bass_kernel_spmd(
        nc, in_maps, list(range(NCORES)), **kwargs
    )
    _CACHE["last_exec_time_ns"] = res.exec_time_ns

    b_out = np.asarray(b_out, dtype=np.float32)
    out = np.empty((B, T, D), dtype=np.float32)
    for b in range(B):
        acc = res.results[4 * b]["out"].astype(np.float32)
        for hg in range(1, 4):
            acc = acc + res.results[4 * b + hg]["out"]
        out[b] = acc + b_out[None, :]
    return out


# revision 30
# speedup vs baseline: 1.0203x; 1.0104x over previous
"""Trainium2 Bass kernel for nn_Attention_55233279426826 (block-causal attention).

Reference computation (per batch b):
    xn = LayerNorm(x[b]) * gamma + beta
    q,k,v = split(xn @ w_qkv), 12 heads x 64
    attn  = softmax(block-causal-masked(q k^T / 8))
    out[b] = (attn v) @ w_out + b_out

Sharding (8 cores): batch (2) x head-group (4, 3 heads each).  Each core gets
its batch's x, the w_qkv columns and w_out rows of its 3 heads, and produces a
partial [2048, 768] output.  Host sums the 4 head-group partials per batch and
adds b_out.  gamma is folded into w_qkv on host; beta@w_qkv is a host-computed
per-channel bias added at QKV psum eviction.  Weights ship as bf16.

Per-core device program:
  1. LayerNorm stats in [token, dim] layout (bn_stats/bn_aggr), apply
     (x - mu) * rstd on DVE -> bf16 xn.
  2. xn -> xnT [768, 2048] via XBAR DMA transposes (off the PE).
  3. qkvT [576, 2048] = w_qkv^T @ xnT on PE; head strips laid out so the two
     heads of a score pair sit on opposite partition halves (auto row-tiling:
     two 64x128 score matmuls run concurrently in the PE array).  Head 2's
     q/k strips are duplicated to the hi partition half via SBUF DMA so odd-J
     score matmuls pair with even-J ones.
  4. v re-transposed to [keys, 64] via XBAR DMA, augmented with a ones column
     (index 64) so A@V also produces softmax denominators in psum row 64.
  5. Scores S_T[j, q] per 128-key block J / 512-query chunk c, exp on ACT
     (bounded scores, no max pass), masked corners zeroed, A@V into psum.
     1/den via DVE reciprocal + GPSIMD partition_broadcast (no PE involved),
     divided out during psum->SBUF eviction.
  6. Attention outputs packed into two [128, T] tiles ([h0;h1] and [h2;0]) so
     the out-projection runs as K=128 full-array matmuls; streamed to DRAM.
  7. QKV/LN/transpose work for token group g+1 is interleaved into attention
     chunk c=g so PE, ACT, DVE and DMA all stay busy.
"""

import contextlib
import ctypes
import math
import os
import sys
import types

import numpy as np

B = 2
T = 2048
D = 768
NPATCH = 64
HEADS = 12
DH = 64
NH = 3          # heads per core
CH = 3 * NH * DH  # 576 qkv channels per core
LN_EPS = 1e-5
NCORES = 8

_CACHE = {}


def _install_axon_hooks_shim():
    """This image's antenv lacks axon_hooks; synthesize it so that
    run_bass_kernel_spmd(trace=True) finds the NTFF profile hook instead of
    crashing on import.  Safe no-op if profiling symbols are unavailable."""
    if "antenv.axon_hooks" in sys.modules:
        return
    mod = types.ModuleType("antenv.axon_hooks")
    _hook = [None]
    mod.set_axon_ntff_profile_hook = lambda h: _hook.__setitem__(0, h)
    mod.get_axon_ntff_profile_hook = lambda: _hook[0]
    sys.modules["antenv.axon_hooks"] = mod
    try:
        lib = ctypes.CDLL("/opt/axon/libaxon_pjrt.so")
        if not hasattr(lib, "axon_start_nrt_profile"):
            return
        lib.axon_start_nrt_profile.argtypes = [
            ctypes.POINTER(ctypes.c_int64),
            ctypes.c_size_t,
        ]
        lib.axon_start_nrt_profile.restype = ctypes.c_int64
        lib.axon_stop_nrt_profile.argtypes = [ctypes.c_char_p]
        lib.axon_stop_nrt_profile.restype = ctypes.c_int64

        @contextlib.contextmanager
        def _hook_cm(output_dir, device_ids):
            import jax

            jax.devices()
            if device_ids:
                ids = (ctypes.c_int64 * len(device_ids))(*device_ids)
                rc = lib.axon_start_nrt_profile(ids, len(device_ids))
            else:
                rc = lib.axon_start_nrt_profile(None, 0)
            if rc != 0:
                raise RuntimeError(f"axon_start_nrt_profile rc={rc}")
            try:
                yield
            finally:
                n = lib.axon_stop_nrt_profile(str(output_dir).encode())
                print(f"profile: {n} file(s) -> {output_dir}", file=sys.stderr)

        mod.set_axon_ntff_profile_hook(_hook_cm)
    except OSError:
        pass


def _install_drain_split():
    """The walrus build in this container accepts only ONE sync wait per
    CTRL(drain) instruction; Tile's tail drain carries several.  Split the
    waits across a chain of drains."""
    import bass_rust
    import concourse.tile as tile
    from concourse.vector_clock import ScopedClock

    if getattr(tile.TileContext, "_drain_split_installed", False):
        return

    def _drain_and_barrier(self, tick_clock, wait_clock):
        nc = self.nc
        drain_inst = nc.sync.drain()
        wait_clock.add_sem_waits(
            drain_inst.ins, ScopedClock({None: tick_clock.global_clock})
        )
        si = drain_inst.ins.sync_info
        if si is not None:
            waits = list(si.on_wait)
            if len(waits) > 1:
                si.on_wait = waits[:1]
                for w in waits[1:]:
                    extra = nc.sync.drain()
                    extra.ins.sync_info = bass_rust.SyncInfo(
                        on_wait=[w], on_update=[]
                    )
        nc.all_engine_barrier()
        popped = nc._tile_sem_poison_stack.pop()
        assert popped is self._sem_poison
        nc.clear_and_free_semaphores(list(self.sems.allocated().values()))
        nc.all_engine_barrier()

    tile.TileContext._drain_and_barrier = _drain_and_barrier

    # Generic pass: walrus here allows 1 sync wait per instruction; move
    # extra waits onto nofuse NOPs inserted just before, on the same engine.
    from concourse import mybir

    orig_lower = tile.TileContext._lower_ordered_insts

    def _lower_split(self, ordered):
        for insts in ordered.values():
            new = []
            for inst in insts:
                si = getattr(inst, "sync_info", None)
                eng = getattr(inst, "engine", None)
                if si is not None and eng is not None:
                    waits = list(si.on_wait)
                    if len(waits) > 1:
                        movable = [w for w in waits
                                   if getattr(w, "sync_type", "") == "semaphore"]
                        keep = [w for w in waits if w not in movable]
                        if not keep:
                            keep = [movable.pop()]
                        for k, w in enumerate(movable):
                            nop = mybir.InstNoOp(
                                name=f"{inst.name}-wsplit{k}",
                                sync_info=mybir.SyncInfo(
                                    on_wait=[w], on_update=[]
                                ),
                                bass_nofuse=True,
                                engine=eng,
                            )
                            new.append(nop)
                        inst.sync_info = mybir.SyncInfo(
                            on_wait=keep, on_update=list(si.on_update)
                        )
                new.append(inst)
            insts[:] = new
        return orig_lower(self, ordered)

    tile.TileContext._lower_ordered_insts = _lower_split
    tile.TileContext._drain_split_installed = True


# qkvT row layout over six [128, T] tiles (64-row strips):
# t0 = [q0; q1], t1 = [k0; k1], t2 = [q2; v0], t3 = [k2; v1],
# t4 = [v2; k2copy], t5 = [--; q2copy]
# q and k of the same head share a partition offset (matmul operands must have
# equal base partitions); the head-2 hi copies land via SBUF->SBUF DMA.
Q_LOC = [(0, 0), (0, 64), (2, 0)]
K_LOC = [(1, 0), (1, 64), (3, 0)]
V_LOC = [(2, 64), (3, 64), (4, 0)]
Q2C = (5, 64)
K2C = (4, 64)
# host column order of the permuted per-core w_qkv (64-col segments)
SEG_ORDER = [("q", 0), ("q", 1), ("k", 0), ("k", 1), ("q", 2), ("v", 0),
             ("k", 2), ("v", 1), ("v", 2)]

C_CHUNKS = [(0, 128), (128, 128), (256, 128), (384, 128), (512, 64)]


def build_nc():
    import concourse.bass as bass
    import concourse.tile as tile
    from concourse import mybir

    _install_drain_split()

    f32 = mybir.dt.float32
    bf16 = mybir.dt.bfloat16
    AF = mybir.ActivationFunctionType
    Alu = mybir.AluOpType

    debug = bool(int(os.environ.get("KERNEL_DEBUG", "0")))
    nc = bass.Bass()
    x_d = nc.dram_tensor("x", [T, D], f32, kind="ExternalInput")
    wqkv_d = nc.dram_tensor("wqkv", [D, CH], bf16, kind="ExternalInput")
    woutp_d = nc.dram_tensor("woutp", [256, D], bf16, kind="ExternalInput")
    bw_d = nc.dram_tensor("bw", [640], f32, kind="ExternalInput")
    ident_d = nc.dram_tensor("ident", [128, 128], bf16, kind="ExternalInput")
    out_d = nc.dram_tensor("out", [T, D], f32, kind="ExternalOutput")
    if debug:
        dbg_qkvT_d = nc.dram_tensor("dbg_qkvT", [6, 128, T], bf16,
                                    kind="ExternalOutput")
        dbg_vaug_d = nc.dram_tensor("dbg_vaug", [NH, 128, 16, 128], bf16,
                                    kind="ExternalOutput")
        dbg_ocat_d = nc.dram_tensor("dbg_ocat", [2, 128, T], bf16,
                                    kind="ExternalOutput")
        dbg_den_d = nc.dram_tensor("dbg_den", [4, NH, 1, 512], f32,
                                   kind="ExternalOutput")
        dbg_rec_d = nc.dram_tensor("dbg_rec", [4, NH, 1, 512], f32,
                                   kind="ExternalOutput")

    with contextlib.ExitStack() as ctx:
        ctx.enter_context(
            nc.allow_low_precision(reason="bf16 PE inputs are intentional")
        )
        tc = ctx.enter_context(tile.TileContext(nc))
        consts = ctx.enter_context(tc.tile_pool(name="consts", bufs=1))
        wpool = ctx.enter_context(tc.tile_pool(name="w", bufs=1))
        qkvT_pool = ctx.enter_context(tc.tile_pool(name="qkvT", bufs=1))
        vaug_pool = ctx.enter_context(tc.tile_pool(name="vaug", bufs=1))
        ocat_pool = ctx.enter_context(tc.tile_pool(name="ocat", bufs=1))
        xpool = ctx.enter_context(tc.tile_pool(name="xin", bufs=1))
        xn_pool = ctx.enter_context(tc.tile_pool(name="xn", bufs=1))
        xnT_pool = ctx.enter_context(tc.tile_pool(name="xnT", bufs=1))
        io_pool = ctx.enter_context(tc.tile_pool(name="io", bufs=3))
        stats = ctx.enter_context(tc.tile_pool(name="stats", bufs=4))
        # bufs=2: group g+1's exp (emitted before group g's deferred A@V)
        # must not alias the pt tiles that A@V still reads
        pt_pool = ctx.enter_context(tc.tile_pool(name="pt", bufs=2))
        rec_pool = ctx.enter_context(tc.tile_pool(name="rec", bufs=2))
        tmp_pool = ctx.enter_context(tc.tile_pool(name="tmp", bufs=2))
        # PSUM banks: 2 (qkv/out-proj shared, tags mm0/mm1) + 3 (scores)
        # + 3 (attn out, tags ot0-2) = 8
        mm_ps = ctx.enter_context(tc.tile_pool(name="mm_ps", bufs=1, space="PSUM"))
        st_ps = ctx.enter_context(tc.tile_pool(name="st_ps", bufs=3, space="PSUM"))
        ot_ps = ctx.enter_context(tc.tile_pool(name="ot_ps", bufs=1, space="PSUM"))

        id_bf = consts.tile([128, 128], bf16, tag="idbf")
        nc.sync.dma_start(id_bf, ident_d[:, :])
        eps_t = consts.tile([128, 1], f32, tag="eps")
        nc.vector.memset(eps_t, LN_EPS)
        ones_t = consts.tile([128, DH], bf16, tag="ones")
        nc.vector.memset(ones_t.bitcast(bf16), 1.0)

        # DMA order: group-0 x tiles first (LN gates the pipeline), then
        # weights (QKV needs them by ~15us), then the rest of x.  Everything
        # split 4-way so no tile's arrival is bound to one ~20GB/s queue.
        x_sb = [xpool.tile([128, D], f32, tag=f"x{i}", name=f"xx{i}")
                for i in range(16)]

        def dma_x(i):
            for q in range(4):
                nc.sync.dma_start(
                    x_sb[i][:, 192 * q : 192 * (q + 1)],
                    x_d[128 * i : 128 * (i + 1), 192 * q : 192 * (q + 1)],
                )

        for i in range(4):
            dma_x(i)
        bw_sb = consts.tile([128, 5], f32, tag="bw")
        nc.sync.dma_start(bw_sb, bw_d[:].rearrange("(a p) -> p a", p=128))
        w_sb = []
        for j in range(6):
            wt = wpool.tile([128, CH], bf16, tag=f"w{j}", name=f"w{j}")
            for q in range(4):
                nc.sync.dma_start(
                    wt[:, 144 * q : 144 * (q + 1)],
                    wqkv_d[128 * j : 128 * (j + 1), 144 * q : 144 * (q + 1)],
                )
            w_sb.append(wt)
        woutp = []
        for p in range(2):
            wo = wpool.tile([128, D], bf16, tag=f"wo{p}", name=f"wo{p}")
            for q in range(2):
                nc.sync.dma_start(
                    wo[:, 384 * q : 384 * (q + 1)],
                    woutp_d[128 * p : 128 * (p + 1), 384 * q : 384 * (q + 1)],
                )
            woutp.append(wo)

        # PE warmup: throwaway matmuls on an uninitialized tile (output
        # never read) get HAM past the cold window while DMAs are in flight
        wu_t = consts.tile([128, 128], bf16, tag="wu")
        nc.vector.memset(wu_t.bitcast(bf16), 0.25)
        wu_ps = st_ps.tile([128, 512], f32, tag="st", name="st")
        # spans the LN startup window (~14us) so HAM is warm when the
        # transposes/QKV arrive; the PE is otherwise idle here
        for r in range(200):
            nc.tensor.matmul(wu_ps[:, :128], wu_t, wu_t,
                             start=(r == 0), stop=(r == 199))

        qkvT = [qkvT_pool.tile([128, T], bf16, tag=f"qkvT{i}", name=f"qkvT{i}")
                for i in range(6)]
        # inner stride padded to 128 elems: XBAR transpose dst offsets stay
        # 256B-aligned; ones column at index 64, A@V lhsT reads [:, J, 0:65]
        vaug = [vaug_pool.tile([128, 16, 128], bf16, tag=f"va{h}", name=f"va{h}")
                for h in range(NH)]
        for h in range(NH):
            nc.gpsimd.memset(vaug[h][:, :, DH : DH + 1].bitcast(bf16), 1.0)
        # two [128, T] attention-output tiles: pair1 = [h0; h1], pair2 = [h2; 0]
        ocat = [ocat_pool.tile([128, T], bf16, tag=f"oc{p}", name=f"oc{p}")
                for p in range(2)]
        nc.gpsimd.memset(ocat[1][64:128, :].bitcast(bf16), 0.0)

        xn_tiles = {}

        def emit_ln(g, u):
            i = 4 * g + u
            xt = x_sb[i]
            st = stats.tile([128, 3, 6], f32, tag="bnst", name="bnst")
            for s in range(3):
                nc.vector.bn_stats(st[:, s, :], xt[:, 256 * s : 256 * (s + 1)])
            mv = stats.tile([128, 2], f32, tag="mv", name="mv")
            nc.vector.bn_aggr(mv, st)
            rstd = stats.tile([128, 1], f32, tag="rstd", name="rstd")
            nc.scalar.activation(rstd, mv[:, 1:2], AF.Sqrt, bias=eps_t)
            nc.vector.reciprocal(rstd, rstd)
            xn_t = xn_pool.tile([128, D], bf16, tag=f"xn{i % 8}", name=f"xn{i % 8}")
            nc.vector.tensor_scalar(
                out=xn_t,
                in0=xt,
                scalar1=mv[:, 0:1],
                scalar2=rstd,
                op0=Alu.subtract,
                op1=Alu.mult,
            )
            xn_tiles[i] = xn_t

        def emit_xT(g, j):
            # PE transpose into the shared score psum ring, then evict
            ps = st_ps.tile([128, 512], bf16, tag="st", name="st")
            for u in range(4):
                i = 4 * g + u
                nc.tensor.transpose(
                    ps[:, 128 * u : 128 * (u + 1)],
                    xn_tiles[i][:, 128 * j : 128 * (j + 1)],
                    id_bf,
                )
            dst = xnT[j][:, 512 * g : 512 * (g + 1)]
            if g == 0 or j % 2 == 0:
                nc.scalar.copy(dst, ps)
            else:
                nc.vector.tensor_copy(dst, ps)

        def emit_qkv(g, ci):
            clo, csz = C_CHUNKS[ci]
            pq = mm_ps.tile([128, 512], f32, tag=f"mm{ci % 2}", name=f"mm{ci % 2}")
            halves = ((0, 256), (256, 256)) if g == 0 else ((0, 512),)
            for hlo, hsz in halves:
                for j in range(6):
                    nc.tensor.matmul(
                        pq[:csz, hlo : hlo + hsz],
                        w_sb[j][:, clo : clo + csz],
                        xnT[j][:, 512 * g + hlo : 512 * g + hlo + hsz],
                        start=(j == 0),
                        stop=(j == 5),
                    )
            nc.vector.tensor_scalar_add(
                qkvT[ci][:csz, 512 * g : 512 * (g + 1)],
                in0=pq[:csz, :],
                scalar1=bw_sb[:csz, ci : ci + 1],
            )

        def emit_vT(g, h):
            tI, ro = V_LOC[h]
            idsl = id_bf[ro : ro + 64, ro : ro + 64]
            ps = st_ps.tile([128, 512], bf16, tag="st", name="st")
            for u in range(4):
                J = 4 * g + u
                nc.tensor.transpose(
                    ps[:, 64 * u : 64 * (u + 1)],
                    qkvT[tI][ro : ro + 64, 128 * J : 128 * (J + 1)],
                    idsl,
                )
            nc.vector.tensor_copy(
                vaug[h][:, 4 * g : 4 * (g + 1), 0:DH],
                ps[:, 0:256].rearrange("p (u d) -> p u d", u=4),
            )

        def emit_dup(g):
            cols = slice(512 * g, 512 * (g + 1))
            qt, qo = Q_LOC[2]
            kt, ko = K_LOC[2]
            nc.sync.dma_start(qkvT[Q2C[0]][64:128, cols], qkvT[qt][qo : qo + 64, cols])
            nc.sync.dma_start(qkvT[K2C[0]][64:128, cols], qkvT[kt][ko : ko + 64, cols])

        xnT = [xnT_pool.tile([128, T], bf16, tag=f"xnT{j}", name=f"xnT{j}")
               for j in range(6)]

        def grp_thunks(g):
            th = []
            if g > 0:
                # x tiles for this group stream in just-in-time so the DMA
                # queues stay clear for latency-critical transfers
                th.append(lambda: [dma_x(4 * g + u) for u in range(4)])
            for u in range(4):
                th.append(lambda u=u: emit_ln(g, u))
            for j in range(6):
                th.append(lambda j=j: emit_xT(g, j))
            for ci in range(4):
                th.append(lambda ci=ci: emit_qkv(g, ci))
            th.append(lambda: emit_dup(g))
            th.append(lambda: emit_qkv(g, 4))
            for h in range(NH):
                th.append(lambda h=h: emit_vT(g, h))
            return th

        # ---- group 0 up front
        for th in grp_thunks(0):
            th()

        # ---- attention: chunks c = 0..3 of 512 queries; J-pair groups with
        # score row-tile pairing; grp c+1 QKV work interleaved between groups.
        def finalize_act(c, otp, h):
            # 1/den = exp(-ln(den)) on ACT; emitted at group top so it beats
            # the group's six exps into the ACT queue
            recf = rec_pool.tile([128, 512], f32, tag="recf", name="recf")
            nc.scalar.activation(recf[64:65, :], otp[h][64:65, :], AF.Ln)
            rec = rec_pool.tile([128, 512], bf16, tag="rec", name="rec")
            nc.scalar.activation(rec[64:65, :], recf[64:65, :], AF.Exp,
                                 scale=-1.0)
            return rec

        def finalize_one(c, otp, h, rec=None):
            if True:
                if rec is None:
                    rec = finalize_act(c, otp, h)
                # broadcast across 64 partitions via a K=1 matmul (bf16
                # stream); st psum ring reused for the broadcast output
                bcp = st_ps.tile([64, 512], f32, tag="st", name="st")
                nc.tensor.matmul(
                    bcp, ones_t[64:65, :], rec[64:65, :], start=True, stop=True
                )
                rec64 = rec_pool.tile([64, 512], f32, tag="rec64", name="rec64")
                nc.vector.tensor_copy(rec64[:], bcp)
                if debug:
                    dent = tmp_pool.tile([128, 512], f32, tag="dent", name="dent")
                    nc.vector.tensor_copy(dent[64:65, :], otp[h][64:65, :])
                    nc.sync.dma_start(dbg_den_d[c, h], dent[64:65, :])
                    nc.sync.dma_start(dbg_rec_d[c, h], rec64[0:1, :])
                if h == 0:
                    nc.vector.tensor_mul(
                        ocat[0][0:64, 512 * c : 512 * (c + 1)],
                        otp[h][0:DH, :],
                        rec64[:],
                    )
                elif h == 2:
                    nc.vector.tensor_mul(
                        ocat[1][0:64, 512 * c : 512 * (c + 1)],
                        otp[h][0:DH, :],
                        rec64[:],
                    )
                else:
                    tmp = tmp_pool.tile([64, 512], bf16, tag="tmp1", name="tmp1")
                    nc.vector.tensor_mul(tmp[:], otp[h][0:DH, :], rec64[:])
                    nc.sync.dma_start(
                        ocat[0][64:128, 512 * c : 512 * (c + 1)], tmp[:]
                    )

        def finalize(c, otp):
            for h in range(NH):
                finalize_one(c, otp, h)

        def emit_oproj(t):
            ot_sb = io_pool.tile([128, D], f32, tag="osb", name="osb")
            for eh in range(2):
                opp = mm_ps.tile([128, 512], f32, tag=f"mm{eh}", name=f"mm{eh}")
                nc.tensor.matmul(
                    opp[:, :384],
                    ocat[0][:, 128 * t : 128 * (t + 1)],
                    woutp[0][:, 384 * eh : 384 * (eh + 1)],
                    start=True,
                    stop=False,
                )
                nc.tensor.matmul(
                    opp[:, :384],
                    ocat[1][:, 128 * t : 128 * (t + 1)],
                    woutp[1][:, 384 * eh : 384 * (eh + 1)],
                    start=False,
                    stop=True,
                )
                nc.vector.tensor_copy(
                    ot_sb[:, 384 * eh : 384 * (eh + 1)], opp[:, :384]
                )
            nc.sync.dma_start(out_d[128 * t : 128 * (t + 1), :], ot_sb)

        deferred = None
        for c in range(4):
            otp = [ot_ps.tile([DH + 1, 512], f32, tag=f"ot{h}", name=f"ot{h}")
                   for h in range(NH)]
            nJ = 4 * c + 4
            ngroups = nJ // 2
            thunks = grp_thunks(c + 1) if c < 3 else []
            tpop = 0
            pending = []
            oproj_todo = []

            def emit_avs(avs):
                for (h, J, s0, n, pt) in avs:
                    nc.tensor.matmul(
                        otp[h][:, s0:512],
                        vaug[h][:, J, 0 : DH + 1],
                        pt[:, :n],
                        start=(J == 0),
                        stop=(J == nJ - 1),
                    )

            for gi in range(ngroups):
                J0, J1 = 2 * gi, 2 * gi + 1
                frec = None
                if deferred is not None and gi < 3:
                    frec = finalize_act(deferred[0], deferred[1], gi)
                prm = []  # (h, J, use_hi_copy)
                prm.append((0, J0, False))
                prm.append((1, J0, False))
                prm.append((2, J0, False))
                prm.append((2, J1, True))
                prm.append((0, J1, False))
                prm.append((1, J1, False))
                avs = []
                for pi, (h, J, hic) in enumerate(prm):
                    s0 = max(0, 128 * J - 512 * c)
                    n = 512 - s0
                    q0 = 512 * c + s0
                    if hic:
                        qt, qo = Q2C
                        kt, ko = K2C
                    else:
                        qt, qo = Q_LOC[h]
                        kt, ko = K_LOC[h]
                    stp = st_ps.tile([128, 512], f32, tag="st", name="st")
                    nc.tensor.matmul(
                        stp[:, :n],
                        qkvT[kt][ko : ko + 64, 128 * J : 128 * (J + 1)],
                        qkvT[qt][qo : qo + 64, q0 : q0 + n],
                        start=True,
                        stop=True,
                    )
                    pt = pt_pool.tile([128, 512], bf16, tag=f"pt{len(avs) % 6}",
                                      name="pt")
                    nc.scalar.activation(
                        pt[:, :n], stp[:, :n], AF.Exp,
                        scale=float(DH) ** -0.5,
                    )
                    if J >= 4 * c:
                        nc.vector.memset(pt[64:128, 0:64].bitcast(bf16), 0.0)
                    avs.append((h, J, s0, n, pt))
                # re-sort AVs into J order (J0's three heads then J1's)
                avs.sort(key=lambda a: (a[1], a[0]))
                pending.append(avs)
                if len(pending) > 1:
                    emit_avs(pending.pop(0))
                if deferred is not None and gi < 3:
                    finalize_one(deferred[0], deferred[1], gi, rec=frec)
                    if gi == 2 or gi == ngroups - 1:
                        oproj_todo = list(
                            range(4 * deferred[0], 4 * deferred[0] + 4)
                        )
                        deferred = None
                # interleave one deferred out-proj token tile per group
                if oproj_todo:
                    emit_oproj(oproj_todo.pop(0))
                # interleave next group's LN/transpose/QKV thunks
                want = math.ceil(len(thunks) * (gi + 1) / ngroups)
                while tpop < want:
                    thunks[tpop]()
                    tpop += 1
            while pending:
                emit_avs(pending.pop(0))
            while oproj_todo:
                emit_oproj(oproj_todo.pop(0))
            deferred = (c, otp)
        finalize(*deferred)
        for t in range(12, 16):
            emit_oproj(t)

        if debug:
            for i in range(6):
                nc.sync.dma_start(dbg_qkvT_d[i], qkvT[i][:])
            for h in range(NH):
                nc.sync.dma_start(dbg_vaug_d[h], vaug[h][:])
            for p in range(2):
                nc.sync.dma_start(dbg_ocat_d[p], ocat[p][:])

    return nc


def shard_inputs(x, gamma, beta, w_qkv, w_out, b_out):
    """Full inputs -> list of 8 per-core input dicts (host-side weight prep)."""
    import ml_dtypes

    bfloat16 = ml_dtypes.bfloat16
    x = np.ascontiguousarray(np.asarray(x, dtype=np.float32))
    gamma = np.asarray(gamma, dtype=np.float32)
    beta = np.asarray(beta, dtype=np.float32)
    w_qkv = np.asarray(w_qkv, dtype=np.float32)
    w_out = np.asarray(w_out, dtype=np.float32)
    in_maps = []
    for g in range(NCORES):
        b = g // 4
        hg = g % 4
        heads = [3 * hg + h for h in range(NH)]
        segs = []
        for kind, h in SEG_ORDER:
            hh = heads[h]
            base = {"q": 0, "k": D, "v": 2 * D}[kind]
            segs.append(w_qkv[:, base + 64 * hh : base + 64 * (hh + 1)])
        wqkv_g = np.ascontiguousarray(np.concatenate(segs, axis=1))
        bw_g = beta @ wqkv_g  # [CH] f32, raw weights
        bw_pack = np.zeros(640, dtype=np.float32)
        bw_pack[:CH] = bw_g
        wqkv_bf = (wqkv_g * gamma[:, None]).astype(bfloat16)
        wout_g = w_out[64 * heads[0] : 64 * (heads[-1] + 1), :]
        woutp = np.zeros((256, D), dtype=np.float32)
        woutp[0:128] = wout_g[0:128]
        woutp[128:192] = wout_g[128:192]
        in_maps.append(
            {
                "x": x[b],
                "wqkv": np.ascontiguousarray(wqkv_bf),
                "woutp": woutp.astype(bfloat16),
                "bw": bw_pack,
                "ident": np.eye(128, dtype=np.float32).astype(bfloat16),
            }
        )
    return in_maps


def kernel(x, gamma, beta, w_qkv, w_out, b_out):
    _install_axon_hooks_shim()
    from concourse import bass_utils

    if "nc" not in _CACHE:
        _CACHE["nc"] = build_nc()
    nc = _CACHE["nc"]

    in_maps = shard_inputs(x, gamma, beta, w_qkv, w_out, b_out)
    trace = bool(int(os.environ.get("KERNEL_TRACE", "0")))
    kwargs = {}
    if trace:
        kwargs["trace"] = True
        tmpdir = os.environ.get("KERNEL_TRACE_DIR")
        if tmpdir:
            kwargs["tmpdir"] = tmpdir
        # artifact upload needs external storage; keep the trace local
        bass_utils.upload_artifacts = lambda d: d
    res = bass_utils.run_bass_kernel_spmd(
        nc, in_maps, list(range(NCORES)), **kwargs
    )
    _CACHE["last_exec_time_ns"] = res.exec_time_ns

    b_out = np.asarray(b_out, dtype=np.float32)
    out = np.empty((B, T, D), dtype=np.float32)
    for b in range(B):
        acc = res.results[4 * b]["out"].astype(np.float32)
        for hg in range(1, 4):
            acc = acc + res.results[4 * b + hg]["out"]
        out[b] = acc + b_out[None, :]
    return out
